# revision 41
# baseline (speedup 1.0000x reference)
"""BFP-quantized 3x3 conv (nn_BFConv2d) on 8 TRN2 NeuronCores.

Two-invocation structure (the BFP group grid is global over the flat
tensor, so each sample's quantized slab starts at a per-(sample,core)
phase pre = start mod 36; re-chunking by pre must happen host-side,
forcing quantize and conv into separate NEFF executions):

  inv1 QUANT: per core, 4 samples. Host supplies bf16-cast (RNE)
    group-aligned windows [128, 6300]. GpSimd computes per-group abs-max
    (groups are bf16 so the max is exact and its exponent equals the f32
    exponent), Vector applies the magic-number snap
        q = (x + M) - M,  M = 1.5 * 2^16 * exp2_bits(absmax)
    (round-half-even onto the BFP lattice; lattice points are <=9
    significant bits so bf16 holds q exactly). DMA in/out ~12.9MB/core.

  inv2 CONV: rho-paired matmul scheme at 75% PE utilization: PSUM
    partitions = 64 outch x 2 output-row-parity, K = 64 inch x 2
    input-row-offsets (dense 128x128 weights, host-built), 6 matmuls
    (2 K-chunks x 3 dw taps) of N=452 per 8-row tile. Input is a
    host-built duplicated padded layout [128, 12884] bf16 per sample
    (partitions 64-127 = same image shifted one row) with 113-strided
    rows sharing single zero pad columns. Output written bf16 in a
    partition-major layout, host de-interleaves rows and casts to f32.
"""

import os
import sys
from contextlib import ExitStack

import numpy as np

sys.path.insert(0, "/opt/trn_rl_repo")

import ml_dtypes  # noqa: E402
import concourse.bacc as bacc  # noqa: E402
import concourse.mybir as mybir  # noqa: E402
import concourse.tile as tile  # noqa: E402

F32 = mybir.dt.float32
BF16 = mybir.dt.bfloat16
I16 = mybir.dt.int16
NPBF16 = ml_dtypes.bfloat16

N_CORES = 8
B = 32                      # batch
C = 64                      # channels (in == out)
H = W = 112
SAMPLE = C * H * W          # 802816 elems per sample
GS = 36                     # BFP group size
GPP = 175                   # groups per partition in the quantize window
QCOLS = GPP * GS            # 6300
QWIN = 128 * QCOLS          # 806400 elems: covers a sample + phase slack
T = 12884                   # conv tile cols: 1 guard + 114*113 + 1 spare
MAGIC_MUL = 98304.0         # 1.5 * 2^16: exp2(e)*this == 1.5*2^23*2^(e-7)

_cache = {}
last_exec_ns = {}
last_results = {}


def _ensure_snap_op():
    """Register a custom DVE op BFP_SNAP_ANT: out = (in0 + in1) - in1."""
    import concourse.dve_ops as dops
    if getattr(dops, "_BFP_SNAP_ANT", None) is not None:
        return dops._BFP_SNAP_ANT
    from concourse.dve_spec import Spec, Src0, Src1, lower as spec_lower
    from concourse.dve_uop import DveOpSpec

    def _snap_ref(in0, in1, s0, s1, imm2):
        a = in0.astype(np.float32)
        b = np.broadcast_to(in1.astype(np.float32), in1.shape).reshape(a.shape)
        return (a + b) - b

    spec = Spec(body=(Src0 + Src1) - Src1, reference=_snap_ref)
    op = dops.DveOp("BFP_SNAP_ANT", spec, subdim=False, uops_sha={})
    idx = max(dops._SUB_OPCODE_FOR_NAME.values()) + 1
    assert idx < 0x20
    dops.OPS.append(op)
    dops.CUSTOM_DVE_SPECS["BFP_SNAP_ANT"] = spec
    dops._SUB_OPCODE_FOR_NAME["BFP_SNAP_ANT"] = idx
    for ver in ("v3", "v4"):
        try:
            s = DveOpSpec(name=op.name, opcode=idx,
                          uops=spec_lower(spec, ver=ver), rd1_en=True)
            op.uops_sha[ver] = s.sha(ver)
        except Exception:
            pass
    dops._BFP_SNAP_ANT = op
    return op


def _trace_enabled():
    return os.environ.get("BFP_TRACE") == "1"


def _install_trace_shim():
    """Provide antenv.axon_hooks (NTFF profiling hook) if the image lacks it."""
    import types
    import ctypes
    import contextlib
    try:
        from antenv.axon_hooks import get_axon_ntff_profile_hook  # noqa: F401
        return
    except ImportError:
        pass
    so_path = "/opt/axon/libaxon_pjrt.so"
    if not os.path.exists(so_path):
        return
    lib = ctypes.CDLL(so_path)
    if not hasattr(lib, "axon_start_nrt_profile"):
        return
    lib.axon_start_nrt_profile.argtypes = [ctypes.POINTER(ctypes.c_int64),
                                           ctypes.c_size_t]
    lib.axon_start_nrt_profile.restype = ctypes.c_int64
    lib.axon_stop_nrt_profile.argtypes = [ctypes.c_char_p]
    lib.axon_stop_nrt_profile.restype = ctypes.c_int64

    @contextlib.contextmanager
    def _hook(output_dir, device_ids):
        import jax
        jax.devices()
        if device_ids:
            ids = (ctypes.c_int64 * len(device_ids))(*device_ids)
            rc = lib.axon_start_nrt_profile(ids, len(device_ids))
        else:
            rc = lib.axon_start_nrt_profile(None, 0)
        if rc != 0:
            raise RuntimeError(f"axon_start_nrt_profile rc={rc}")
        try:
            yield
        finally:
            n = lib.axon_stop_nrt_profile(str(output_dir).encode())
            print(f"profile: {n} ntff file(s) -> {output_dir}", file=sys.stderr)

    mod = types.ModuleType("antenv.axon_hooks")
    state = {"hook": _hook}
    mod.get_axon_ntff_profile_hook = lambda: state["hook"]
    mod.set_axon_ntff_profile_hook = lambda h: state.update(hook=h)
    sys.modules["antenv.axon_hooks"] = mod
    import antenv
    antenv.axon_hooks = mod
    from concourse import bass_utils as bu
    bu.upload_artifacts = lambda d: str(d)  # no egress from this container


ICOLS = 36 * 176            # 6336: interleaved window, col = s*176 + g


def build_quant():
    """Interleaved-layout quant: host delivers windows as [s:36, g:176]
    (col = s*176 + g, g=175 is zero pad) so the per-group abs-max becomes a
    cascade of contiguous builtin tensor_tensor max ops (2x-eligible), with
    abs on ScalarE and the magic snap as a custom DVE op."""
    snap = _ensure_snap_op()
    nc = bacc.Bacc(None)
    xin = nc.declare_dram_parameter("xin", [4, 128, ICOLS], BF16, isOutput=False)
    qx = nc.declare_dram_parameter("qx", [4, 128, ICOLS], BF16, isOutput=True)

    MAX = mybir.AluOpType.max
    with tile.TileContext(nc) as tc:
        with ExitStack() as ctx:
            xpool = ctx.enter_context(tc.tile_pool(name="xp", bufs=4))
            apool = ctx.enter_context(tc.tile_pool(name="ap", bufs=2))
            qpool = ctx.enter_context(tc.tile_pool(name="qp", bufs=2))
            spool = ctx.enter_context(tc.tile_pool(name="small", bufs=4))

            def cascade_full(xa, mtag):
                # one scratch tile for all levels: same-engine chaining needs
                # no cross-buffer bookkeeping (offsets kept 4B-aligned)
                u = spool.tile([128, 6160], BF16, tag="casc", name="u")
                nc.vector.tensor_tensor(u[:, 0:2816], xa[:, 0:2816],
                                        xa[:, 2816:5632], MAX)
                nc.vector.tensor_tensor(u[:, 2816:4224], u[:, 0:1408],
                                        u[:, 1408:2816], MAX)
                nc.vector.tensor_tensor(u[:, 4224:4928], u[:, 2816:3520],
                                        u[:, 3520:4224], MAX)
                nc.vector.tensor_tensor(u[:, 4928:5632], u[:, 4224:4928],
                                        xa[:, 5632:6336], MAX)
                nc.vector.tensor_tensor(u[:, 5632:5984], u[:, 4928:5280],
                                        u[:, 5280:5632], MAX)
                m = spool.tile([128, 176], BF16, tag=mtag, name="m")
                nc.vector.tensor_tensor(m[:], u[:, 5632:5808],
                                        u[:, 5808:5984], MAX)
                return m

            def cascade_half(xa, lo, mtag):
                # 18-slab max tree (16-tree + leftover pair + merge)
                v = xa[:, lo:lo + 3168]
                t = spool.tile([128, 2816], BF16, tag="casc_h", name="t")
                nc.vector.tensor_tensor(t[:, 0:1408], v[:, 0:1408],
                                        v[:, 1408:2816], MAX)
                nc.vector.tensor_tensor(t[:, 1408:2112], t[:, 0:704],
                                        t[:, 704:1408], MAX)
                nc.vector.tensor_tensor(t[:, 2112:2464], t[:, 1408:1760],
                                        t[:, 1760:2112], MAX)
                nc.vector.tensor_tensor(t[:, 2464:2640], t[:, 2112:2288],
                                        t[:, 2288:2464], MAX)
                nc.vector.tensor_tensor(t[:, 2640:2816], v[:, 2816:2992],
                                        v[:, 2992:3168], MAX)
                m = spool.tile([128, 176], BF16, tag=mtag, name="m")
                nc.vector.tensor_tensor(m[:], t[:, 2464:2640],
                                        t[:, 2640:2816], MAX)
                return m

            def cascade_quarter(xa, lo, mtag):
                # 9-slab max tree (8-tree + leftover slab + merge)
                v = xa[:, lo:lo + 1584]
                t = spool.tile([128, 1408], BF16, tag="casc_q", name="t")
                nc.vector.tensor_tensor(t[:, 0:704], v[:, 0:704],
                                        v[:, 704:1408], MAX)
                nc.vector.tensor_tensor(t[:, 704:1056], t[:, 0:352],
                                        t[:, 352:704], MAX)
                nc.vector.tensor_tensor(t[:, 1056:1232], t[:, 704:880],
                                        t[:, 880:1056], MAX)
                m = spool.tile([128, 176], BF16, tag=mtag, name="m")
                nc.vector.tensor_tensor(m[:], t[:, 1056:1232],
                                        v[:, 1408:1584], MAX)
                return m

            def magic(m):
                mi = spool.tile([128, 176], I16, tag="mi", name="mi")
                nc.vector.tensor_scalar(mi[:], m[:].bitcast(I16), 0x7F80, None,
                                        op0=mybir.AluOpType.bitwise_and)
                mf = spool.tile([128, 176], BF16, tag="mf", name="mf")
                nc.vector.tensor_scalar(mf[:], mi[:].bitcast(BF16), MAGIC_MUL,
                                        None, op0=mybir.AluOpType.mult)
                return mf

            def absmax_phase(j):
                xb = xpool.tile([128, ICOLS], BF16, tag="xb", name="xb")
                xa = apool.tile([128, ICOLS], BF16, tag="xa", name="xa")
                ABS = mybir.ActivationFunctionType.Abs
                if j == 0:
                    # quarter/half-granular ramp: abs/cascade start after a
                    # quarter of the first load
                    nc.sync.dma_start(xb[:, 0:1584], xin[j, :, 0:1584])
                    nc.sync.dma_start(xb[:, 1584:3168], xin[j, :, 1584:3168])
                    nc.sync.dma_start(xb[:, 3168:ICOLS], xin[j, :, 3168:ICOLS])
                    nc.scalar.activation(xa[:, 0:1584], xb[:, 0:1584], ABS)
                    mq1 = cascade_quarter(xa, 0, "mq1")
                    nc.scalar.activation(xa[:, 1584:3168], xb[:, 1584:3168],
                                         ABS)
                    mq2 = cascade_quarter(xa, 1584, "mq2")
                    nc.scalar.activation(xa[:, 3168:ICOLS], xb[:, 3168:ICOLS],
                                         ABS)
                    mh2 = cascade_half(xa, 3168, "mh2")
                    mq = spool.tile([128, 176], BF16, tag="mq", name="mq")
                    nc.vector.tensor_tensor(mq[:], mq1[:], mq2[:], MAX)
                    m = spool.tile([128, 176], BF16, tag="m", name="m")
                    nc.vector.tensor_tensor(m[:], mq[:], mh2[:], MAX)
                else:
                    nc.sync.dma_start(xb[:], xin[j])
                    nc.scalar.activation(xa[:], xb[:], ABS)
                    m = cascade_full(xa, "m")
                return xb, magic(m)

            def snap_phase(j, xb, mf):
                q = qpool.tile([128, ICOLS], BF16, tag="q", name="q")
                if j == 3:
                    # quarter-granular tail: stores overlap remaining snaps
                    for lo in range(0, ICOLS, 1584):
                        mb = mf[:].unsqueeze(-2).broadcast_to([128, 9, 176])
                        nc.vector._custom_dve(
                            snap, out=q[:, lo:lo + 1584].rearrange(
                                "p (s g) -> p s g", g=176),
                            in0=xb[:, lo:lo + 1584].rearrange(
                                "p (s g) -> p s g", g=176), in1=mb)
                        nc.scalar.dma_start(qx[j, :, lo:lo + 1584],
                                            q[:, lo:lo + 1584])
                else:
                    mb = mf[:].unsqueeze(-2).broadcast_to([128, GS, 176])
                    nc.vector._custom_dve(
                        snap, out=q[:].rearrange("p (s g) -> p s g", g=176),
                        in0=xb[:].rearrange("p (s g) -> p s g", g=176), in1=mb)
                    nc.scalar.dma_start(qx[j], q[:])

            for j in range(0, 4, 2):
                a0 = absmax_phase(j)
                a1 = absmax_phase(j + 1)
                snap_phase(j, *a0)
                snap_phase(j + 1, *a1)
    nc.compile()
    return nc


def build_conv():
    nc = bacc.Bacc(None)
    qx4 = nc.declare_dram_parameter("qx4", [4, 128, T], BF16, isOutput=False)
    wblk = nc.declare_dram_parameter("wblk", [128, 6 * 128], BF16, isOutput=False)
    bias2 = nc.declare_dram_parameter("bias2", [128], F32, isOutput=False)
    out = nc.declare_dram_parameter("out", [4, 128, 6272], BF16, isOutput=True)

    # per-block max col = 904*(last_tile) + 1018; chunk loads gate block starts
    XCHUNKS = [0, 1930, 3760, 7360, 10976, T]
    BLOCKS = [(0, 2), (2, 2), (4, 4), (8, 4), (12, 2)]

    with tile.TileContext(nc) as tc:
        with ExitStack() as ctx:
            consts = ctx.enter_context(tc.tile_pool(name="consts", bufs=1))
            xpool = ctx.enter_context(tc.tile_pool(name="x", bufs=2))
            opool = ctx.enter_context(tc.tile_pool(name="o", bufs=2))
            psum = ctx.enter_context(tc.tile_pool(name="ps", bufs=2,
                                                  space="PSUM"))

            wsb = consts.tile([128, 6 * 128], BF16)
            nc.sync.dma_start(wsb[:], wblk[:])
            bias_sb = consts.tile([128, 1], F32)
            nc.sync.dma_start(bias_sb[:], bias2[:, None])

            # PE warmup: dummy matmuls on a zeroed tile while the first input
            # chunks load, so HAM reaches 8/8 before the real stream starts
            warm = consts.tile([128, 512], BF16)
            nc.gpsimd.memset(warm[:], 0.0)
            wps = psum.tile([128, 512], F32, tag="ps0", name="wps")
            for w in range(7):
                nc.tensor.matmul(wps[:], warm[:, 0:128], warm[:],
                                 start=(w == 0), stop=(w == 6))

            for s in range(4):
                xt = xpool.tile([128, T], BF16, tag="xt")
                for a, b in zip(XCHUNKS, XCHUNKS[1:]):
                    nc.sync.dma_start(xt[:, a:b], qx4[s, :, a:b])
                osb = opool.tile([128, 6272], BF16, tag="osb")
                for tb, nt in BLOCKS:
                    pss = [psum.tile([128, 512], F32, tag=f"ps{i}",
                                     name=f"ps{i}") for i in range(nt)]
                    for ci in range(6):
                        ch, dw = divmod(ci, 3)
                        lhs = wsb[:, ci * 128:(ci + 1) * 128]
                        for i in range(nt):
                            t = tb + i
                            for h in range(2):
                                base = 904 * t + 226 * ch + dw + 452 * h
                                rhs = xt[:, base:base + 452].rearrange(
                                    "p (j u) -> p j u", u=226)[:, :, 0:113]
                                nc.tensor.matmul(
                                    pss[i][:, 226 * h:226 * h + 226], lhs,
                                    rhs, start=(ci == 0 and h == 0),
                                    stop=(ci == 5 and h == 1))
                    for i in range(nt):
                        t = tb + i
                        nc.vector.tensor_scalar(
                            osb[:, t * 448:(t + 1) * 448].rearrange(
                                "p (j w) -> p j w", j=4),
                            pss[i][:, 0:452].rearrange(
                                "p (j u) -> p j u", j=4)[:, :, 1:113],
                            bias_sb[:, 0:1], None, op0=mybir.AluOpType.add)
                    if tb == 4:
                        nc.scalar.dma_start(out[s, :, 0:3584],
                                            osb[:, 0:3584])
                    elif tb == 12:
                        nc.scalar.dma_start(out[s, :, 3584:5376],
                                            osb[:, 3584:5376])
                nc.scalar.dma_start(out[s, :, 5376:6272], osb[:, 5376:6272])
    nc.compile()
    return nc


def _bfp_quantize_host(x):
    """Exact numpy replication of reference bfp_quantize (f32 semantics)."""
    flat = x.reshape(-1).astype(np.float32)
    n = flat.shape[0]
    pad = (-n) % GS
    f = np.concatenate([flat, np.zeros(pad, np.float32)]).reshape(-1, GS)
    m = np.max(np.abs(f), axis=1, keepdims=True).astype(np.float32)
    safe = np.where(m > 0, m, np.ones_like(m))
    e = np.floor(np.log2(safe)).astype(np.float32)
    scale = np.exp2(e - 7).astype(np.float32)
    q = (np.round(f / scale) * scale).astype(np.float32)
    q = np.where(m > 0, q, np.zeros_like(q))
    return q.reshape(-1)[:n].reshape(x.shape)


def _pack_weights(weight, bias):
    """wblk6 [128, 768] bf16 + bias128 [128] f32 (host-exact BFP quant)."""
    wq = _bfp_quantize_host(np.asarray(weight, np.float32))
    wb = np.zeros((128, 6, 128), np.float32)
    for ci in range(6):
        klow = -1 if ci < 3 else 1
        dw = ci % 3
        for ki in range(2):
            for rho in range(2):
                dh = (klow + ki) - rho + 1
                if 0 <= dh <= 2:
                    wb[64 * ki:64 * ki + 64, ci, 64 * rho:64 * rho + 64] = \
                        wq[:, :, dh, dw].T
    bias128 = np.concatenate([np.asarray(bias, np.float32)] * 2)
    return wb.reshape(128, 768).astype(NPBF16), bias128


def _shard_inputs(x):
    """Per-core bf16 group-aligned interleaved windows + per-sample phases."""
    xf = np.concatenate([np.asarray(x, np.float32).reshape(-1),
                         np.zeros(QWIN, np.float32)])
    xb = xf.astype(NPBF16)
    in_maps = []
    pres = []
    for k in range(N_CORES):
        core_pre = []
        xin = np.zeros((4, 128, 36, 176), NPBF16)
        for j in range(4):
            start = (4 * k + j) * SAMPLE
            g0 = (start // GS) * GS
            core_pre.append(start - g0)
            xin[j, :, :, 0:GPP] = (xb[g0:g0 + QWIN]
                                   .reshape(128, GPP, GS).transpose(0, 2, 1))
        in_maps.append({"xin": xin.reshape(4, 128, ICOLS)})
        pres.append(core_pre)
    return in_maps, pres


def _pack_conv_inputs(qx, core_pre, wblk6, bias128):
    """qx [4,128,6300] bf16 (window layout) -> conv in_map for one core."""
    dup = np.zeros((4, 128, T), NPBF16)
    for j in range(4):
        pre = core_pre[j]
        qw = (np.asarray(qx[j]).reshape(128, GS, 176)[:, :, 0:GPP]
              .transpose(0, 2, 1).reshape(-1))
        qs = qw[pre:pre + SAMPLE].reshape(C, H, W)
        Bq = np.zeros((C, 114, 113), NPBF16)
        Bq[:, 1:113, 1:113] = qs
        dup[j, :64, 1:12883] = Bq.reshape(C, 12882)
    dup[:, 64:, :T - 113] = dup[:, :64, 113:]
    return {"qx4": dup, "wblk": wblk6, "bias2": bias128}


def _unpack_out(od):
    """[4,128,6272] bf16 partition-major -> [4,64,112,112] f32."""
    return np.asarray(od).reshape(4, 2, 64, 14, 4, 112) \
        .transpose(0, 2, 3, 4, 1, 5).reshape(4, C, H, W).astype(np.float32)


def kernel(x, weight, bias):
    from concourse.bass_utils import run_bass_kernel_spmd

    if "quant" not in _cache:
        _cache["quant"] = build_quant()
    if "conv" not in _cache:
        _cache["conv"] = build_conv()

    core_ids = list(range(N_CORES))
    trace = _trace_enabled()
    if trace:
        _install_trace_shim()

    in_maps, pres = _shard_inputs(x)
    resA = run_bass_kernel_spmd(_cache["quant"], in_maps, core_ids, trace=trace)
    last_exec_ns["quant"] = resA.exec_time_ns
    last_results["quant"] = resA

    wblk6, bias128 = _pack_weights(weight, bias)
    in_maps_b = [
        _pack_conv_inputs(resA.results[k]["qx"], pres[k], wblk6, bias128)
        for k in range(N_CORES)
    ]
    resB = run_bass_kernel_spmd(_cache["conv"], in_maps_b, core_ids, trace=trace)
    last_exec_ns["conv"] = resB.exec_time_ns
    last_results["conv"] = resB

    out = np.concatenate(
        [_unpack_out(resB.results[k]["out"]) for k in range(N_CORES)], axis=0)
    return out


# revision 43
# speedup vs baseline: 1.0037x; 1.0037x over previous
"""BFP-quantized 3x3 conv (nn_BFConv2d) on 8 TRN2 NeuronCores.

Two-invocation structure (the BFP group grid is global over the flat
tensor, so each sample's quantized slab starts at a per-(sample,core)
phase pre = start mod 36; re-chunking by pre must happen host-side,
forcing quantize and conv into separate NEFF executions):

  inv1 QUANT: per core, 4 samples. Host supplies bf16-cast (RNE)
    group-aligned windows [128, 6300]. GpSimd computes per-group abs-max
    (groups are bf16 so the max is exact and its exponent equals the f32
    exponent), Vector applies the magic-number snap
        q = (x + M) - M,  M = 1.5 * 2^16 * exp2_bits(absmax)
    (round-half-even onto the BFP lattice; lattice points are <=9
    significant bits so bf16 holds q exactly). DMA in/out ~12.9MB/core.

  inv2 CONV: rho-paired matmul scheme at 75% PE utilization: PSUM
    partitions = 64 outch x 2 output-row-parity, K = 64 inch x 2
    input-row-offsets (dense 128x128 weights, host-built), 6 matmuls
    (2 K-chunks x 3 dw taps) of N=452 per 8-row tile. Input is a
    host-built duplicated padded layout [128, 12884] bf16 per sample
    (partitions 64-127 = same image shifted one row) with 113-strided
    rows sharing single zero pad columns. Output written bf16 in a
    partition-major layout, host de-interleaves rows and casts to f32.
"""

import os
import sys
from contextlib import ExitStack

import numpy as np

sys.path.insert(0, "/opt/trn_rl_repo")

import ml_dtypes  # noqa: E402
import concourse.bacc as bacc  # noqa: E402
import concourse.mybir as mybir  # noqa: E402
import concourse.tile as tile  # noqa: E402

F32 = mybir.dt.float32
BF16 = mybir.dt.bfloat16
I16 = mybir.dt.int16
NPBF16 = ml_dtypes.bfloat16

N_CORES = 8
B = 32                      # batch
C = 64                      # channels (in == out)
H = W = 112
SAMPLE = C * H * W          # 802816 elems per sample
GS = 36                     # BFP group size
GPP = 175                   # groups per partition in the quantize window
QCOLS = GPP * GS            # 6300
QWIN = 128 * QCOLS          # 806400 elems: covers a sample + phase slack
T = 12885                   # conv tile cols: 1 guard + 114*113 + 2 spare
MAGIC_MUL = 98304.0         # 1.5 * 2^16: exp2(e)*this == 1.5*2^23*2^(e-7)

_cache = {}
last_exec_ns = {}
last_results = {}


def _ensure_snap_op():
    """Register a custom DVE op BFP_SNAP_ANT: out = (in0 + in1) - in1."""
    import concourse.dve_ops as dops
    if getattr(dops, "_BFP_SNAP_ANT", None) is not None:
        return dops._BFP_SNAP_ANT
    from concourse.dve_spec import Spec, Src0, Src1, lower as spec_lower
    from concourse.dve_uop import DveOpSpec

    def _snap_ref(in0, in1, s0, s1, imm2):
        a = in0.astype(np.float32)
        b = np.broadcast_to(in1.astype(np.float32), in1.shape).reshape(a.shape)
        return (a + b) - b

    spec = Spec(body=(Src0 + Src1) - Src1, reference=_snap_ref)
    op = dops.DveOp("BFP_SNAP_ANT", spec, subdim=False, uops_sha={})
    idx = max(dops._SUB_OPCODE_FOR_NAME.values()) + 1
    assert idx < 0x20
    dops.OPS.append(op)
    dops.CUSTOM_DVE_SPECS["BFP_SNAP_ANT"] = spec
    dops._SUB_OPCODE_FOR_NAME["BFP_SNAP_ANT"] = idx
    for ver in ("v3", "v4"):
        try:
            s = DveOpSpec(name=op.name, opcode=idx,
                          uops=spec_lower(spec, ver=ver), rd1_en=True)
            op.uops_sha[ver] = s.sha(ver)
        except Exception:
            pass
    dops._BFP_SNAP_ANT = op
    return op


def _trace_enabled():
    return os.environ.get("BFP_TRACE") == "1"


def _install_trace_shim():
    """Provide antenv.axon_hooks (NTFF profiling hook) if the image lacks it."""
    import types
    import ctypes
    import contextlib
    try:
        from antenv.axon_hooks import get_axon_ntff_profile_hook  # noqa: F401
        return
    except ImportError:
        pass
    so_path = "/opt/axon/libaxon_pjrt.so"
    if not os.path.exists(so_path):
        return
    lib = ctypes.CDLL(so_path)
    if not hasattr(lib, "axon_start_nrt_profile"):
        return
    lib.axon_start_nrt_profile.argtypes = [ctypes.POINTER(ctypes.c_int64),
                                           ctypes.c_size_t]
    lib.axon_start_nrt_profile.restype = ctypes.c_int64
    lib.axon_stop_nrt_profile.argtypes = [ctypes.c_char_p]
    lib.axon_stop_nrt_profile.restype = ctypes.c_int64

    @contextlib.contextmanager
    def _hook(output_dir, device_ids):
        import jax
        jax.devices()
        if device_ids:
            ids = (ctypes.c_int64 * len(device_ids))(*device_ids)
            rc = lib.axon_start_nrt_profile(ids, len(device_ids))
        else:
            rc = lib.axon_start_nrt_profile(None, 0)
        if rc != 0:
            raise RuntimeError(f"axon_start_nrt_profile rc={rc}")
        try:
            yield
        finally:
            n = lib.axon_stop_nrt_profile(str(output_dir).encode())
            print(f"profile: {n} ntff file(s) -> {output_dir}", file=sys.stderr)

    mod = types.ModuleType("antenv.axon_hooks")
    state = {"hook": _hook}
    mod.get_axon_ntff_profile_hook = lambda: state["hook"]
    mod.set_axon_ntff_profile_hook = lambda h: state.update(hook=h)
    sys.modules["antenv.axon_hooks"] = mod
    import antenv
    antenv.axon_hooks = mod
    from concourse import bass_utils as bu
    bu.upload_artifacts = lambda d: str(d)  # no egress from this container


ICOLS = 36 * 176            # 6336: interleaved window, col = s*176 + g


def build_quant():
    """Interleaved-layout quant: host delivers windows as [s:36, g:176]
    (col = s*176 + g, g=175 is zero pad) so the per-group abs-max becomes a
    cascade of contiguous builtin tensor_tensor max ops (2x-eligible), with
    abs on ScalarE and the magic snap as a custom DVE op."""
    snap = _ensure_snap_op()
    nc = bacc.Bacc(None)
    xin = nc.declare_dram_parameter("xin", [4, 128, ICOLS], BF16, isOutput=False)
    qx = nc.declare_dram_parameter("qx", [4, 128, ICOLS], BF16, isOutput=True)

    MAX = mybir.AluOpType.max
    with tile.TileContext(nc) as tc:
        with ExitStack() as ctx:
            xpool = ctx.enter_context(tc.tile_pool(name="xp", bufs=4))
            apool = ctx.enter_context(tc.tile_pool(name="ap", bufs=2))
            qpool = ctx.enter_context(tc.tile_pool(name="qp", bufs=2))
            spool = ctx.enter_context(tc.tile_pool(name="small", bufs=4))

            def cascade_full(xa, mtag):
                # one scratch tile for all levels: same-engine chaining needs
                # no cross-buffer bookkeeping (offsets kept 4B-aligned)
                u = spool.tile([128, 6160], BF16, tag="casc", name="u")
                nc.vector.tensor_tensor(u[:, 0:2816], xa[:, 0:2816],
                                        xa[:, 2816:5632], MAX)
                nc.vector.tensor_tensor(u[:, 2816:4224], u[:, 0:1408],
                                        u[:, 1408:2816], MAX)
                nc.vector.tensor_tensor(u[:, 4224:4928], u[:, 2816:3520],
                                        u[:, 3520:4224], MAX)
                nc.vector.tensor_tensor(u[:, 4928:5632], u[:, 4224:4928],
                                        xa[:, 5632:6336], MAX)
                nc.vector.tensor_tensor(u[:, 5632:5984], u[:, 4928:5280],
                                        u[:, 5280:5632], MAX)
                m = spool.tile([128, 176], BF16, tag=mtag, name="m")
                nc.vector.tensor_tensor(m[:], u[:, 5632:5808],
                                        u[:, 5808:5984], MAX)
                return m

            def cascade_half(xa, lo, mtag):
                # 18-slab max tree (16-tree + leftover pair + merge)
                v = xa[:, lo:lo + 3168]
                t = spool.tile([128, 2816], BF16, tag="casc_h", name="t")
                nc.vector.tensor_tensor(t[:, 0:1408], v[:, 0:1408],
                                        v[:, 1408:2816], MAX)
                nc.vector.tensor_tensor(t[:, 1408:2112], t[:, 0:704],
                                        t[:, 704:1408], MAX)
                nc.vector.tensor_tensor(t[:, 2112:2464], t[:, 1408:1760],
                                        t[:, 1760:2112], MAX)
                nc.vector.tensor_tensor(t[:, 2464:2640], t[:, 2112:2288],
                                        t[:, 2288:2464], MAX)
                nc.vector.tensor_tensor(t[:, 2640:2816], v[:, 2816:2992],
                                        v[:, 2992:3168], MAX)
                m = spool.tile([128, 176], BF16, tag=mtag, name="m")
                nc.vector.tensor_tensor(m[:], t[:, 2464:2640],
                                        t[:, 2640:2816], MAX)
                return m

            def cascade_quarter(xa, lo, mtag):
                # 9-slab max tree (8-tree + leftover slab + merge)
                v = xa[:, lo:lo + 1584]
                t = spool.tile([128, 1408], BF16, tag="casc_q", name="t")
                nc.vector.tensor_tensor(t[:, 0:704], v[:, 0:704],
                                        v[:, 704:1408], MAX)
                nc.vector.tensor_tensor(t[:, 704:1056], t[:, 0:352],
                                        t[:, 352:704], MAX)
                nc.vector.tensor_tensor(t[:, 1056:1232], t[:, 704:880],
                                        t[:, 880:1056], MAX)
                m = spool.tile([128, 176], BF16, tag=mtag, name="m")
                nc.vector.tensor_tensor(m[:], t[:, 1056:1232],
                                        v[:, 1408:1584], MAX)
                return m

            def magic(m):
                mi = spool.tile([128, 176], I16, tag="mi", name="mi")
                nc.vector.tensor_scalar(mi[:], m[:].bitcast(I16), 0x7F80, None,
                                        op0=mybir.AluOpType.bitwise_and)
                mf = spool.tile([128, 176], BF16, tag="mf", name="mf")
                nc.vector.tensor_scalar(mf[:], mi[:].bitcast(BF16), MAGIC_MUL,
                                        None, op0=mybir.AluOpType.mult)
                return mf

            def absmax_phase(j):
                xb = xpool.tile([128, ICOLS], BF16, tag="xb", name="xb")
                xa = apool.tile([128, ICOLS], BF16, tag="xa", name="xa")
                ABS = mybir.ActivationFunctionType.Abs
                if j == 0:
                    # quarter/half-granular ramp: abs/cascade start after a
                    # quarter of the first load
                    nc.sync.dma_start(xb[:, 0:1584], xin[j, :, 0:1584])
                    nc.sync.dma_start(xb[:, 1584:3168], xin[j, :, 1584:3168])
                    nc.sync.dma_start(xb[:, 3168:ICOLS], xin[j, :, 3168:ICOLS])
                    nc.scalar.activation(xa[:, 0:1584], xb[:, 0:1584], ABS)
                    mq1 = cascade_quarter(xa, 0, "mq1")
                    nc.scalar.activation(xa[:, 1584:3168], xb[:, 1584:3168],
                                         ABS)
                    mq2 = cascade_quarter(xa, 1584, "mq2")
                    nc.scalar.activation(xa[:, 3168:ICOLS], xb[:, 3168:ICOLS],
                                         ABS)
                    mh2 = cascade_half(xa, 3168, "mh2")
                    mq = spool.tile([128, 176], BF16, tag="mq", name="mq")
                    nc.vector.tensor_tensor(mq[:], mq1[:], mq2[:], MAX)
                    m = spool.tile([128, 176], BF16, tag="m", name="m")
                    nc.vector.tensor_tensor(m[:], mq[:], mh2[:], MAX)
                else:
                    nc.sync.dma_start(xb[:], xin[j])
                    nc.scalar.activation(xa[:], xb[:], ABS)
                    m = cascade_full(xa, "m")
                return xb, magic(m)

            def snap_phase(j, xb, mf):
                q = qpool.tile([128, ICOLS], BF16, tag="q", name="q")
                if j == 3:
                    # quarter-granular tail: stores overlap remaining snaps
                    for lo in range(0, ICOLS, 1584):
                        mb = mf[:].unsqueeze(-2).broadcast_to([128, 9, 176])
                        nc.vector._custom_dve(
                            snap, out=q[:, lo:lo + 1584].rearrange(
                                "p (s g) -> p s g", g=176),
                            in0=xb[:, lo:lo + 1584].rearrange(
                                "p (s g) -> p s g", g=176), in1=mb)
                        nc.scalar.dma_start(qx[j, :, lo:lo + 1584],
                                            q[:, lo:lo + 1584])
                else:
                    mb = mf[:].unsqueeze(-2).broadcast_to([128, GS, 176])
                    nc.vector._custom_dve(
                        snap, out=q[:].rearrange("p (s g) -> p s g", g=176),
                        in0=xb[:].rearrange("p (s g) -> p s g", g=176), in1=mb)
                    nc.scalar.dma_start(qx[j], q[:])

            for j in range(0, 4, 2):
                a0 = absmax_phase(j)
                a1 = absmax_phase(j + 1)
                snap_phase(j, *a0)
                snap_phase(j + 1, *a1)
    nc.compile()
    return nc


def build_conv():
    nc = bacc.Bacc(None)
    qx4 = nc.declare_dram_parameter("qx4", [4, 128, T], BF16, isOutput=False)
    wblk = nc.declare_dram_parameter("wblk", [128, 6 * 128], BF16, isOutput=False)
    bias2 = nc.declare_dram_parameter("bias2", [128], F32, isOutput=False)
    out = nc.declare_dram_parameter("out", [4, 128, 6272], BF16, isOutput=True)

    # per-block max col = 904*(last_tile) + 1018; chunk loads gate block starts
    XCHUNKS = [0, 1930, 3760, 7360, 10976, T]
    BLOCKS = [(0, 2), (2, 2), (4, 4), (8, 4), (12, 2)]

    with tile.TileContext(nc) as tc:
        with ExitStack() as ctx:
            consts = ctx.enter_context(tc.tile_pool(name="consts", bufs=1))
            xpool = ctx.enter_context(tc.tile_pool(name="x", bufs=2))
            opool = ctx.enter_context(tc.tile_pool(name="o", bufs=2))
            psum = ctx.enter_context(tc.tile_pool(name="ps", bufs=2,
                                                  space="PSUM"))

            wsb = consts.tile([128, 6 * 128], BF16)
            nc.sync.dma_start(wsb[:], wblk[:])
            bias_sb = consts.tile([128, 1], F32)
            nc.sync.dma_start(bias_sb[:], bias2[:, None])

            # PE warmup: dummy matmuls on a zeroed tile while the first input
            # chunks load, so HAM reaches 8/8 before the real stream starts
            warm = consts.tile([128, 512], BF16)
            nc.gpsimd.memset(warm[:], 0.0)
            wps = psum.tile([128, 512], F32, tag="ps0", name="wps")
            for w in range(7):
                nc.tensor.matmul(wps[:], warm[:, 0:128], warm[:],
                                 start=(w == 0), stop=(w == 6))

            for s in range(4):
                xt = xpool.tile([128, T], BF16, tag="xt")
                for a, b in zip(XCHUNKS, XCHUNKS[1:]):
                    nc.sync.dma_start(xt[:, a:b], qx4[s, :, a:b])
                osb = opool.tile([128, 6272], BF16, tag="osb")
                for tb, nt in BLOCKS:
                    pss = [psum.tile([128, 512], F32, tag=f"ps{i}",
                                     name=f"ps{i}") for i in range(nt)]
                    for ci in range(6):
                        ch, dw = divmod(ci, 3)
                        lhs = wsb[:, ci * 128:(ci + 1) * 128]
                        for i in range(nt):
                            t = tb + i
                            for h in range(2):
                                base = 904 * t + 226 * ch + dw + 1 + 452 * h
                                rhs = xt[:, base:base + 452].rearrange(
                                    "p (j u) -> p j u", u=226)[:, :, 0:112]
                                nc.tensor.matmul(
                                    pss[i][:, 224 * h:224 * h + 224], lhs,
                                    rhs, start=(ci == 0 and h == 0),
                                    stop=(ci == 5 and h == 1))
                    for i in range(nt):
                        t = tb + i
                        nc.vector.tensor_scalar(
                            osb[:, t * 448:(t + 1) * 448],
                            pss[i][:, 0:448],
                            bias_sb[:, 0:1], None, op0=mybir.AluOpType.add)
                    if tb == 4:
                        nc.scalar.dma_start(out[s, :, 0:3584],
                                            osb[:, 0:3584])
                    elif tb == 12:
                        nc.scalar.dma_start(out[s, :, 3584:5376],
                                            osb[:, 3584:5376])
                nc.scalar.dma_start(out[s, :, 5376:6272], osb[:, 5376:6272])
    nc.compile()
    return nc


def _bfp_quantize_host(x):
    """Exact numpy replication of reference bfp_quantize (f32 semantics)."""
    flat = x.reshape(-1).astype(np.float32)
    n = flat.shape[0]
    pad = (-n) % GS
    f = np.concatenate([flat, np.zeros(pad, np.float32)]).reshape(-1, GS)
    m = np.max(np.abs(f), axis=1, keepdims=True).astype(np.float32)
    safe = np.where(m > 0, m, np.ones_like(m))
    e = np.floor(np.log2(safe)).astype(np.float32)
    scale = np.exp2(e - 7).astype(np.float32)
    q = (np.round(f / scale) * scale).astype(np.float32)
    q = np.where(m > 0, q, np.zeros_like(q))
    return q.reshape(-1)[:n].reshape(x.shape)


def _pack_weights(weight, bias):
    """wblk6 [128, 768] bf16 + bias128 [128] f32 (host-exact BFP quant)."""
    wq = _bfp_quantize_host(np.asarray(weight, np.float32))
    wb = np.zeros((128, 6, 128), np.float32)
    for ci in range(6):
        klow = -1 if ci < 3 else 1
        dw = ci % 3
        for ki in range(2):
            for rho in range(2):
                dh = (klow + ki) - rho + 1
                if 0 <= dh <= 2:
                    wb[64 * ki:64 * ki + 64, ci, 64 * rho:64 * rho + 64] = \
                        wq[:, :, dh, dw].T
    bias128 = np.concatenate([np.asarray(bias, np.float32)] * 2)
    return wb.reshape(128, 768).astype(NPBF16), bias128


def _shard_inputs(x):
    """Per-core bf16 group-aligned interleaved windows + per-sample phases."""
    xf = np.concatenate([np.asarray(x, np.float32).reshape(-1),
                         np.zeros(QWIN, np.float32)])
    xb = xf.astype(NPBF16)
    in_maps = []
    pres = []
    for k in range(N_CORES):
        core_pre = []
        xin = np.zeros((4, 128, 36, 176), NPBF16)
        for j in range(4):
            start = (4 * k + j) * SAMPLE
            g0 = (start // GS) * GS
            core_pre.append(start - g0)
            xin[j, :, :, 0:GPP] = (xb[g0:g0 + QWIN]
                                   .reshape(128, GPP, GS).transpose(0, 2, 1))
        in_maps.append({"xin": xin.reshape(4, 128, ICOLS)})
        pres.append(core_pre)
    return in_maps, pres


def _pack_conv_inputs(qx, core_pre, wblk6, bias128):
    """qx [4,128,6300] bf16 (window layout) -> conv in_map for one core."""
    dup = np.zeros((4, 128, T), NPBF16)
    for j in range(4):
        pre = core_pre[j]
        qw = (np.asarray(qx[j]).reshape(128, GS, 176)[:, :, 0:GPP]
              .transpose(0, 2, 1).reshape(-1))
        qs = qw[pre:pre + SAMPLE].reshape(C, H, W)
        Bq = np.zeros((C, 114, 113), NPBF16)
        Bq[:, 1:113, 1:113] = qs
        dup[j, :64, 1:12883] = Bq.reshape(C, 12882)
    dup[:, 64:, :T - 113] = dup[:, :64, 113:]
    return {"qx4": dup, "wblk": wblk6, "bias2": bias128}


def _unpack_out(od):
    """[4,128,6272] bf16 partition-major -> [4,64,112,112] f32."""
    return np.asarray(od).reshape(4, 2, 64, 14, 4, 112) \
        .transpose(0, 2, 3, 4, 1, 5).reshape(4, C, H, W).astype(np.float32)


def kernel(x, weight, bias):
    from concourse.bass_utils import run_bass_kernel_spmd

    if "quant" not in _cache:
        _cache["quant"] = build_quant()
    if "conv" not in _cache:
        _cache["conv"] = build_conv()

    core_ids = list(range(N_CORES))
    trace = _trace_enabled()
    if trace:
        _install_trace_shim()

    in_maps, pres = _shard_inputs(x)
    resA = run_bass_kernel_spmd(_cache["quant"], in_maps, core_ids, trace=trace)
    last_exec_ns["quant"] = resA.exec_time_ns
    last_results["quant"] = resA

    wblk6, bias128 = _pack_weights(weight, bias)
    in_maps_b = [
        _pack_conv_inputs(resA.results[k]["qx"], pres[k], wblk6, bias128)
        for k in range(N_CORES)
    ]
    resB = run_bass_kernel_spmd(_cache["conv"], in_maps_b, core_ids, trace=trace)
    last_exec_ns["conv"] = resB.exec_time_ns
    last_results["conv"] = resB

    out = np.concatenate(
        [_unpack_out(resB.results[k]["out"]) for k in range(N_CORES)], axis=0)
    return out


# revision 45
# speedup vs baseline: 1.0218x; 1.0179x over previous
"""BFP-quantized 3x3 conv (nn_BFConv2d) on 8 TRN2 NeuronCores.

Two-invocation structure (the BFP group grid is global over the flat
tensor, so each sample's quantized slab starts at a per-(sample,core)
phase pre = start mod 36; re-chunking by pre must happen host-side,
forcing quantize and conv into separate NEFF executions):

  inv1 QUANT: per core, 4 samples. Host supplies bf16-cast (RNE)
    group-aligned windows [128, 6300]. GpSimd computes per-group abs-max
    (groups are bf16 so the max is exact and its exponent equals the f32
    exponent), Vector applies the magic-number snap
        q = (x + M) - M,  M = 1.5 * 2^16 * exp2_bits(absmax)
    (round-half-even onto the BFP lattice; lattice points are <=9
    significant bits so bf16 holds q exactly). DMA in/out ~12.9MB/core.

  inv2 CONV: rho-paired matmul scheme at 75% PE utilization: PSUM
    partitions = 64 outch x 2 output-row-parity, K = 64 inch x 2
    input-row-offsets (dense 128x128 weights, host-built), 6 matmuls
    (2 K-chunks x 3 dw taps) of N=452 per 8-row tile. Input is a
    host-built duplicated padded layout [128, 12884] bf16 per sample
    (partitions 64-127 = same image shifted one row) with 113-strided
    rows sharing single zero pad columns. Output written bf16 in a
    partition-major layout, host de-interleaves rows and casts to f32.
"""

import os
import sys
from contextlib import ExitStack

import numpy as np

sys.path.insert(0, "/opt/trn_rl_repo")

import ml_dtypes  # noqa: E402
import concourse.bacc as bacc  # noqa: E402
import concourse.mybir as mybir  # noqa: E402
import concourse.tile as tile  # noqa: E402

F32 = mybir.dt.float32
BF16 = mybir.dt.bfloat16
I16 = mybir.dt.int16
NPBF16 = ml_dtypes.bfloat16

N_CORES = 8
B = 32                      # batch
C = 64                      # channels (in == out)
H = W = 112
SAMPLE = C * H * W          # 802816 elems per sample
GS = 36                     # BFP group size
GPP = 175                   # groups per partition in the quantize window
QCOLS = GPP * GS            # 6300
QWIN = 128 * QCOLS          # 806400 elems: covers a sample + phase slack
T = 12885                   # conv tile cols: 1 guard + 114*113 + 2 spare
MAGIC_MUL = 98304.0         # 1.5 * 2^16: exp2(e)*this == 1.5*2^23*2^(e-7)

_cache = {}
last_exec_ns = {}
last_results = {}


def _ensure_snap_op():
    """Register a custom DVE op BFP_SNAP_ANT: out = (in0 + in1) - in1."""
    import concourse.dve_ops as dops
    if getattr(dops, "_BFP_SNAP_ANT", None) is not None:
        return dops._BFP_SNAP_ANT
    from concourse.dve_spec import Spec, Src0, Src1, lower as spec_lower
    from concourse.dve_uop import DveOpSpec

    def _snap_ref(in0, in1, s0, s1, imm2):
        a = in0.astype(np.float32)
        b = np.broadcast_to(in1.astype(np.float32), in1.shape).reshape(a.shape)
        return (a + b) - b

    spec = Spec(body=(Src0 + Src1) - Src1, reference=_snap_ref)
    op = dops.DveOp("BFP_SNAP_ANT", spec, subdim=False, uops_sha={})
    idx = max(dops._SUB_OPCODE_FOR_NAME.values()) + 1
    assert idx < 0x20
    dops.OPS.append(op)
    dops.CUSTOM_DVE_SPECS["BFP_SNAP_ANT"] = spec
    dops._SUB_OPCODE_FOR_NAME["BFP_SNAP_ANT"] = idx
    for ver in ("v3", "v4"):
        try:
            s = DveOpSpec(name=op.name, opcode=idx,
                          uops=spec_lower(spec, ver=ver), rd1_en=True)
            op.uops_sha[ver] = s.sha(ver)
        except Exception:
            pass
    dops._BFP_SNAP_ANT = op
    return op


def _trace_enabled():
    return os.environ.get("BFP_TRACE") == "1"


def _install_trace_shim():
    """Provide antenv.axon_hooks (NTFF profiling hook) if the image lacks it."""
    import types
    import ctypes
    import contextlib
    try:
        from antenv.axon_hooks import get_axon_ntff_profile_hook  # noqa: F401
        return
    except ImportError:
        pass
    so_path = "/opt/axon/libaxon_pjrt.so"
    if not os.path.exists(so_path):
        return
    lib = ctypes.CDLL(so_path)
    if not hasattr(lib, "axon_start_nrt_profile"):
        return
    lib.axon_start_nrt_profile.argtypes = [ctypes.POINTER(ctypes.c_int64),
                                           ctypes.c_size_t]
    lib.axon_start_nrt_profile.restype = ctypes.c_int64
    lib.axon_stop_nrt_profile.argtypes = [ctypes.c_char_p]
    lib.axon_stop_nrt_profile.restype = ctypes.c_int64

    @contextlib.contextmanager
    def _hook(output_dir, device_ids):
        import jax
        jax.devices()
        if device_ids:
            ids = (ctypes.c_int64 * len(device_ids))(*device_ids)
            rc = lib.axon_start_nrt_profile(ids, len(device_ids))
        else:
            rc = lib.axon_start_nrt_profile(None, 0)
        if rc != 0:
            raise RuntimeError(f"axon_start_nrt_profile rc={rc}")
        try:
            yield
        finally:
            n = lib.axon_stop_nrt_profile(str(output_dir).encode())
            print(f"profile: {n} ntff file(s) -> {output_dir}", file=sys.stderr)

    mod = types.ModuleType("antenv.axon_hooks")
    state = {"hook": _hook}
    mod.get_axon_ntff_profile_hook = lambda: state["hook"]
    mod.set_axon_ntff_profile_hook = lambda h: state.update(hook=h)
    sys.modules["antenv.axon_hooks"] = mod
    import antenv
    antenv.axon_hooks = mod
    from concourse import bass_utils as bu
    bu.upload_artifacts = lambda d: str(d)  # no egress from this container


ICOLS = 36 * 176            # 6336: interleaved window, col = s*176 + g


def build_quant():
    """Interleaved-layout quant: host delivers windows as [s:36, g:176]
    (col = s*176 + g, g=175 is zero pad) so the per-group abs-max becomes a
    cascade of contiguous builtin tensor_tensor max ops (2x-eligible), with
    abs on ScalarE and the magic snap as a custom DVE op."""
    snap = _ensure_snap_op()
    nc = bacc.Bacc(None)
    xin = nc.declare_dram_parameter("xin", [4, 128, ICOLS], BF16, isOutput=False)
    qx = nc.declare_dram_parameter("qx", [4, 128, ICOLS], BF16, isOutput=True)

    MAX = mybir.AluOpType.max
    with tile.TileContext(nc) as tc:
        with ExitStack() as ctx:
            xpool = ctx.enter_context(tc.tile_pool(name="xp", bufs=4))
            apool = ctx.enter_context(tc.tile_pool(name="ap", bufs=2))
            qpool = ctx.enter_context(tc.tile_pool(name="qp", bufs=2))
            spool = ctx.enter_context(tc.tile_pool(name="small", bufs=4))

            def cascade_full(xa, mtag):
                # one scratch tile for all levels: same-engine chaining needs
                # no cross-buffer bookkeeping (offsets kept 4B-aligned)
                u = spool.tile([128, 6160], BF16, tag="casc", name="u")
                nc.vector.tensor_tensor(u[:, 0:2816], xa[:, 0:2816],
                                        xa[:, 2816:5632], MAX)
                nc.vector.tensor_tensor(u[:, 2816:4224], u[:, 0:1408],
                                        u[:, 1408:2816], MAX)
                nc.vector.tensor_tensor(u[:, 4224:4928], u[:, 2816:3520],
                                        u[:, 3520:4224], MAX)
                nc.vector.tensor_tensor(u[:, 4928:5632], u[:, 4224:4928],
                                        xa[:, 5632:6336], MAX)
                nc.vector.tensor_tensor(u[:, 5632:5984], u[:, 4928:5280],
                                        u[:, 5280:5632], MAX)
                m = spool.tile([128, 176], BF16, tag=mtag, name="m")
                nc.vector.tensor_tensor(m[:], u[:, 5632:5808],
                                        u[:, 5808:5984], MAX)
                return m

            def cascade_half(xa, lo, mtag):
                # 18-slab max tree (16-tree + leftover pair + merge)
                v = xa[:, lo:lo + 3168]
                t = spool.tile([128, 2816], BF16, tag="casc_h", name="t")
                nc.vector.tensor_tensor(t[:, 0:1408], v[:, 0:1408],
                                        v[:, 1408:2816], MAX)
                nc.vector.tensor_tensor(t[:, 1408:2112], t[:, 0:704],
                                        t[:, 704:1408], MAX)
                nc.vector.tensor_tensor(t[:, 2112:2464], t[:, 1408:1760],
                                        t[:, 1760:2112], MAX)
                nc.vector.tensor_tensor(t[:, 2464:2640], t[:, 2112:2288],
                                        t[:, 2288:2464], MAX)
                nc.vector.tensor_tensor(t[:, 2640:2816], v[:, 2816:2992],
                                        v[:, 2992:3168], MAX)
                m = spool.tile([128, 176], BF16, tag=mtag, name="m")
                nc.vector.tensor_tensor(m[:], t[:, 2464:2640],
                                        t[:, 2640:2816], MAX)
                return m

            def cascade_quarter(xa, lo, mtag):
                # 9-slab max tree (8-tree + leftover slab + merge)
                v = xa[:, lo:lo + 1584]
                t = spool.tile([128, 1408], BF16, tag="casc_q", name="t")
                nc.vector.tensor_tensor(t[:, 0:704], v[:, 0:704],
                                        v[:, 704:1408], MAX)
                nc.vector.tensor_tensor(t[:, 704:1056], t[:, 0:352],
                                        t[:, 352:704], MAX)
                nc.vector.tensor_tensor(t[:, 1056:1232], t[:, 704:880],
                                        t[:, 880:1056], MAX)
                m = spool.tile([128, 176], BF16, tag=mtag, name="m")
                nc.vector.tensor_tensor(m[:], t[:, 1056:1232],
                                        v[:, 1408:1584], MAX)
                return m

            def magic(m):
                mi = spool.tile([128, 176], I16, tag="mi", name="mi")
                nc.vector.tensor_scalar(mi[:], m[:].bitcast(I16), 0x7F80, None,
                                        op0=mybir.AluOpType.bitwise_and)
                mf = spool.tile([128, 176], BF16, tag="mf", name="mf")
                nc.vector.tensor_scalar(mf[:], mi[:].bitcast(BF16), MAGIC_MUL,
                                        None, op0=mybir.AluOpType.mult)
                return mf

            def absmax_phase(j):
                xb = xpool.tile([128, ICOLS], BF16, tag="xb", name="xb")
                xa = apool.tile([128, ICOLS], BF16, tag="xa", name="xa")
                ABS = mybir.ActivationFunctionType.Abs
                if j == 0:
                    # quarter/half-granular ramp: abs/cascade start after a
                    # quarter of the first load
                    nc.sync.dma_start(xb[:, 0:1584], xin[j, :, 0:1584])
                    nc.sync.dma_start(xb[:, 1584:3168], xin[j, :, 1584:3168])
                    nc.sync.dma_start(xb[:, 3168:ICOLS], xin[j, :, 3168:ICOLS])
                    nc.scalar.activation(xa[:, 0:1584], xb[:, 0:1584], ABS)
                    mq1 = cascade_quarter(xa, 0, "mq1")
                    nc.scalar.activation(xa[:, 1584:3168], xb[:, 1584:3168],
                                         ABS)
                    mq2 = cascade_quarter(xa, 1584, "mq2")
                    nc.scalar.activation(xa[:, 3168:ICOLS], xb[:, 3168:ICOLS],
                                         ABS)
                    mh2 = cascade_half(xa, 3168, "mh2")
                    mq = spool.tile([128, 176], BF16, tag="mq", name="mq")
                    nc.vector.tensor_tensor(mq[:], mq1[:], mq2[:], MAX)
                    m = spool.tile([128, 176], BF16, tag="m", name="m")
                    nc.vector.tensor_tensor(m[:], mq[:], mh2[:], MAX)
                else:
                    nc.sync.dma_start(xb[:], xin[j])
                    nc.scalar.activation(xa[:], xb[:], ABS)
                    m = cascade_full(xa, "m")
                return xb, magic(m)

            def snap_phase(j, xb, mf):
                q = qpool.tile([128, ICOLS], BF16, tag="q", name="q")
                if j == 3:
                    # quarter-granular tail: stores overlap remaining snaps
                    for lo in range(0, ICOLS, 1584):
                        mb = mf[:].unsqueeze(-2).broadcast_to([128, 9, 176])
                        nc.vector._custom_dve(
                            snap, out=q[:, lo:lo + 1584].rearrange(
                                "p (s g) -> p s g", g=176),
                            in0=xb[:, lo:lo + 1584].rearrange(
                                "p (s g) -> p s g", g=176), in1=mb)
                        nc.scalar.dma_start(qx[j, :, lo:lo + 1584],
                                            q[:, lo:lo + 1584])
                else:
                    mb = mf[:].unsqueeze(-2).broadcast_to([128, GS, 176])
                    nc.vector._custom_dve(
                        snap, out=q[:].rearrange("p (s g) -> p s g", g=176),
                        in0=xb[:].rearrange("p (s g) -> p s g", g=176), in1=mb)
                    nc.scalar.dma_start(qx[j], q[:])

            for j in range(0, 4, 2):
                a0 = absmax_phase(j)
                a1 = absmax_phase(j + 1)
                snap_phase(j, *a0)
                snap_phase(j + 1, *a1)
    nc.compile()
    return nc


def build_conv():
    nc = bacc.Bacc(None)
    qx4 = nc.declare_dram_parameter("qx4", [4, 128, T], BF16, isOutput=False)
    wblk = nc.declare_dram_parameter("wblk", [128, 6 * 128], BF16, isOutput=False)
    bias2 = nc.declare_dram_parameter("bias2", [128], F32, isOutput=False)
    out = nc.declare_dram_parameter("out", [4, 128, 6272], BF16, isOutput=True)

    # per-block max col = 904*(last_tile) + 1018; chunk loads gate block starts
    XCHUNKS = [0, 1026, 1930, 3760, 7360, 10976, T]
    BLOCKS = [(0, 1), (1, 1), (2, 2), (4, 4), (8, 4), (12, 1), (13, 1)]

    with tile.TileContext(nc) as tc:
        with ExitStack() as ctx:
            consts = ctx.enter_context(tc.tile_pool(name="consts", bufs=1))
            xpool = ctx.enter_context(tc.tile_pool(name="x", bufs=2))
            opool = ctx.enter_context(tc.tile_pool(name="o", bufs=2))
            psum = ctx.enter_context(tc.tile_pool(name="ps", bufs=2,
                                                  space="PSUM"))

            wsb = consts.tile([128, 6 * 128], BF16)
            nc.sync.dma_start(wsb[:], wblk[:])
            bias_sb = consts.tile([128, 1], F32)
            nc.sync.dma_start(bias_sb[:], bias2[:, None])

            # PE warmup: dummy matmuls on a zeroed tile while the first input
            # chunks load, so HAM reaches 8/8 before the real stream starts
            warm = consts.tile([128, 512], BF16)
            nc.gpsimd.memset(warm[:], 0.0)
            wps = psum.tile([128, 512], F32, tag="ps0", name="wps")
            for w in range(7):
                nc.tensor.matmul(wps[:], warm[:, 0:128], warm[:],
                                 start=(w == 0), stop=(w == 6))

            for s in range(4):
                xt = xpool.tile([128, T], BF16, tag="xt")
                for a, b in zip(XCHUNKS, XCHUNKS[1:]):
                    nc.sync.dma_start(xt[:, a:b], qx4[s, :, a:b])
                osb = opool.tile([128, 6272], BF16, tag="osb")
                for tb, nt in BLOCKS:
                    pss = [psum.tile([128, 512], F32, tag=f"ps{i}",
                                     name=f"ps{i}") for i in range(nt)]
                    for ci in range(6):
                        ch, dw = divmod(ci, 3)
                        lhs = wsb[:, ci * 128:(ci + 1) * 128]
                        for i in range(nt):
                            t = tb + i
                            for h in range(2):
                                base = 904 * t + 226 * ch + dw + 1 + 452 * h
                                rhs = xt[:, base:base + 452].rearrange(
                                    "p (j u) -> p j u", u=226)[:, :, 0:112]
                                nc.tensor.matmul(
                                    pss[i][:, 224 * h:224 * h + 224], lhs,
                                    rhs, start=(ci == 0 and h == 0),
                                    stop=(ci == 5 and h == 1))
                    for i in range(nt):
                        t = tb + i
                        nc.vector.tensor_scalar(
                            osb[:, t * 448:(t + 1) * 448],
                            pss[i][:, 0:448],
                            bias_sb[:, 0:1], None, op0=mybir.AluOpType.add)
                    if tb == 4:
                        nc.scalar.dma_start(out[s, :, 0:3584],
                                            osb[:, 0:3584])
                    elif tb == 12:
                        nc.scalar.dma_start(out[s, :, 3584:5376],
                                            osb[:, 3584:5376])
                    elif tb == 13:
                        nc.scalar.dma_start(out[s, :, 5376:5824],
                                            osb[:, 5376:5824])
                nc.scalar.dma_start(out[s, :, 5824:6272], osb[:, 5824:6272])
    nc.compile()
    return nc


def _bfp_quantize_host(x):
    """Exact numpy replication of reference bfp_quantize (f32 semantics)."""
    flat = x.reshape(-1).astype(np.float32)
    n = flat.shape[0]
    pad = (-n) % GS
    f = np.concatenate([flat, np.zeros(pad, np.float32)]).reshape(-1, GS)
    m = np.max(np.abs(f), axis=1, keepdims=True).astype(np.float32)
    safe = np.where(m > 0, m, np.ones_like(m))
    e = np.floor(np.log2(safe)).astype(np.float32)
    scale = np.exp2(e - 7).astype(np.float32)
    q = (np.round(f / scale) * scale).astype(np.float32)
    q = np.where(m > 0, q, np.zeros_like(q))
    return q.reshape(-1)[:n].reshape(x.shape)


def _pack_weights(weight, bias):
    """wblk6 [128, 768] bf16 + bias128 [128] f32 (host-exact BFP quant)."""
    wq = _bfp_quantize_host(np.asarray(weight, np.float32))
    wb = np.zeros((128, 6, 128), np.float32)
    for ci in range(6):
        klow = -1 if ci < 3 else 1
        dw = ci % 3
        for ki in range(2):
            for rho in range(2):
                dh = (klow + ki) - rho + 1
                if 0 <= dh <= 2:
                    wb[64 * ki:64 * ki + 64, ci, 64 * rho:64 * rho + 64] = \
                        wq[:, :, dh, dw].T
    bias128 = np.concatenate([np.asarray(bias, np.float32)] * 2)
    return wb.reshape(128, 768).astype(NPBF16), bias128


def _shard_inputs(x):
    """Per-core bf16 group-aligned interleaved windows + per-sample phases."""
    xf = np.concatenate([np.asarray(x, np.float32).reshape(-1),
                         np.zeros(QWIN, np.float32)])
    xb = xf.astype(NPBF16)
    in_maps = []
    pres = []
    for k in range(N_CORES):
        core_pre = []
        xin = np.zeros((4, 128, 36, 176), NPBF16)
        for j in range(4):
            start = (4 * k + j) * SAMPLE
            g0 = (start // GS) * GS
            core_pre.append(start - g0)
            xin[j, :, :, 0:GPP] = (xb[g0:g0 + QWIN]
                                   .reshape(128, GPP, GS).transpose(0, 2, 1))
        in_maps.append({"xin": xin.reshape(4, 128, ICOLS)})
        pres.append(core_pre)
    return in_maps, pres


def _pack_conv_inputs(qx, core_pre, wblk6, bias128):
    """qx [4,128,6300] bf16 (window layout) -> conv in_map for one core."""
    dup = np.zeros((4, 128, T), NPBF16)
    for j in range(4):
        pre = core_pre[j]
        qw = (np.asarray(qx[j]).reshape(128, GS, 176)[:, :, 0:GPP]
              .transpose(0, 2, 1).reshape(-1))
        qs = qw[pre:pre + SAMPLE].reshape(C, H, W)
        Bq = np.zeros((C, 114, 113), NPBF16)
        Bq[:, 1:113, 1:113] = qs
        dup[j, :64, 1:12883] = Bq.reshape(C, 12882)
    dup[:, 64:, :T - 113] = dup[:, :64, 113:]
    return {"qx4": dup, "wblk": wblk6, "bias2": bias128}


def _unpack_out(od):
    """[4,128,6272] bf16 partition-major -> [4,64,112,112] f32."""
    return np.asarray(od).reshape(4, 2, 64, 14, 4, 112) \
        .transpose(0, 2, 3, 4, 1, 5).reshape(4, C, H, W).astype(np.float32)


def kernel(x, weight, bias):
    from concourse.bass_utils import run_bass_kernel_spmd

    if "quant" not in _cache:
        _cache["quant"] = build_quant()
    if "conv" not in _cache:
        _cache["conv"] = build_conv()

    core_ids = list(range(N_CORES))
    trace = _trace_enabled()
    if trace:
        _install_trace_shim()

    in_maps, pres = _shard_inputs(x)
    resA = run_bass_kernel_spmd(_cache["quant"], in_maps, core_ids, trace=trace)
    last_exec_ns["quant"] = resA.exec_time_ns
    last_results["quant"] = resA

    wblk6, bias128 = _pack_weights(weight, bias)
    in_maps_b = [
        _pack_conv_inputs(resA.results[k]["qx"], pres[k], wblk6, bias128)
        for k in range(N_CORES)
    ]
    resB = run_bass_kernel_spmd(_cache["conv"], in_maps_b, core_ids, trace=trace)
    last_exec_ns["conv"] = resB.exec_time_ns
    last_results["conv"] = resB

    out = np.concatenate(
        [_unpack_out(resB.results[k]["out"]) for k in range(N_CORES)], axis=0)
    return out
